# revision 1
# baseline (speedup 1.0000x reference)
"""Trainium2 Bass kernel for nn_CausalTransformer_81776177316304.

Strategy: DP-2 over batch x TP-4 over heads/FFN (groups [0-3], [4-7]).

The thought-structure (nt=2, rtc=512) makes the block-causal mask equivalent,
after de-interleaving rows into [thought-0 | thought-1] halves, to:
  - block A (rows 0..511):   causal-inclusive attention over block A keys
  - block B (rows 512..1023): causal-inclusive attention over block A keys
                              plus a self-attention diagonal term
so attention runs on 512-wide causal blocks with no S x S mask tensor.

Per core: 3 heads, 512 FFN channels. Per layer: AllGather of attention
outputs (channel-sharded) + AllReduce of partial FFN outputs within each
group of 4. All matmuls in fp32r (tf32-like) with fp32 PSUM accumulation.
"""

import numpy as np

import concourse.bass as bass
import concourse.mybir as mybir
import concourse.tile as tile
from concourse import bacc
from concourse.bass_utils import run_bass_kernel_spmd
from concourse.masks import make_identity, make_causal_mask

F32 = mybir.dt.float32
F32R = mybir.dt.float32r
AF = mybir.ActivationFunctionType
ALU = mybir.AluOpType
AX = mybir.AxisListType

S, E, H, L, FF, D = 1024, 768, 12, 4, 2048, 64
NB = S // 2                      # 512: A/B block size
HPC, QKO, VO, FFC = 3, 512, 192, 512  # per-core heads, q|k feats (padded), v feats, ff slice
ET, ST = E // 128, S // 128      # 6 e-tiles, 8 s-tiles
LN_EPS = 1e-5
RG = [[0, 1, 2, 3], [4, 5, 6, 7]]

_NC_CACHE = None
LAST_RESULT = None


def _emit_ln(nc, x_tile, out_ap, stat, sq_tile, epsb):
    """LayerNorm over the 768-wide free dim of x_tile ([128, E], fp32).

    Writes normalized result to out_ap. x_tile is left unmodified.
    rstd computed as exp(-0.5*ln(var+eps)) to stay in the exp/ln ACT table set.
    """
    nsum = stat.tile([128, 1], F32, tag="nsum", bufs=4, name="nsum")
    nc.vector.tensor_reduce(out=nsum[:], in_=x_tile[:], op=ALU.add, axis=AX.X,
                            negate=True)
    nmean = stat.tile([128, 1], F32, tag="nmean", bufs=4, name="nmean")
    nc.vector.tensor_scalar_mul(nmean[:], nsum[:], 1.0 / E)          # = -mu
    ssq = stat.tile([128, 1], F32, tag="ssq", bufs=4, name="ssq")
    nc.scalar.activation(sq_tile[:], x_tile[:], AF.Square, accum_out=ssq[:])
    musq = stat.tile([128, 1], F32, tag="musq", bufs=4, name="musq")
    nc.vector.tensor_mul(musq[:], nmean[:], nmean[:])
    var = stat.tile([128, 1], F32, tag="var", bufs=4, name="var")
    nc.vector.tensor_scalar(out=var[:], in0=ssq[:], scalar1=1.0 / E,
                            scalar2=musq[:], op0=ALU.mult, op1=ALU.subtract)
    lnv = stat.tile([128, 1], F32, tag="lnv", bufs=4, name="lnv")
    nc.scalar.activation(lnv[:], var[:], AF.Ln, bias=epsb[:])
    rstd = stat.tile([128, 1], F32, tag="rstd", bufs=4, name="rstd")
    nc.scalar.activation(rstd[:], lnv[:], AF.Exp, scale=-0.5)
    nb = stat.tile([128, 1], F32, tag="nb", bufs=4, name="nb")
    nc.vector.tensor_mul(nb[:], nmean[:], rstd[:])
    nc.vector.tensor_scalar(out=out_ap, in0=x_tile[:], scalar1=rstd[:],
                            scalar2=nb[:], op0=ALU.mult, op1=ALU.add)


def _build():
    nc = bacc.Bacc("TRN2", target_bir_lowering=False, debug=False, num_devices=8)
    h0 = nc.dram_tensor("h0", [S, E], F32, kind="ExternalInput")
    wqk = nc.dram_tensor("wqk", [L, E, QKO], F32R, kind="ExternalInput")
    wv = nc.dram_tensor("wv", [L, E, VO], F32R, kind="ExternalInput")
    w1 = nc.dram_tensor("w1", [L, E, FFC], F32R, kind="ExternalInput")
    w2 = nc.dram_tensor("w2", [L, FFC, E], F32R, kind="ExternalInput")
    out = nc.dram_tensor("out", [S, E], F32, kind="ExternalOutput")

    from contextlib import ExitStack
    with tile.TileContext(nc) as tc:
        with ExitStack() as ctx:
            const = ctx.enter_context(tc.tile_pool(name="const", bufs=1))
            hpool = ctx.enter_context(tc.tile_pool(name="hpool", bufs=1))
            htpool = ctx.enter_context(tc.tile_pool(name="htpool", bufs=1))
            wpool = ctx.enter_context(tc.tile_pool(name="wpool", bufs=1))
            qkpool = ctx.enter_context(tc.tile_pool(name="qkpool", bufs=1))
            vpool = ctx.enter_context(tc.tile_pool(name="vpool", bufs=1))
            avspool = ctx.enter_context(tc.tile_pool(name="avspool", bufs=6))
            ppool = ctx.enter_context(tc.tile_pool(name="ppool", bufs=4))
            ptpool = ctx.enter_context(tc.tile_pool(name="ptpool", bufs=8))
            aopool = ctx.enter_context(tc.tile_pool(name="aopool", bufs=6))
            ffpool = ctx.enter_context(tc.tile_pool(name="ffpool", bufs=6))
            hidpool = ctx.enter_context(tc.tile_pool(name="hidpool", bufs=1))
            stat = ctx.enter_context(tc.tile_pool(name="stat", bufs=4))
            statp = ctx.enter_context(tc.tile_pool(name="statp", bufs=26))
            psum = ctx.enter_context(tc.tile_pool(name="psum", bufs=2, space="PSUM"))
            dram = ctx.enter_context(tc.tile_pool(name="dram", bufs=2, space="DRAM"))
            ident = const.tile([128, 128], F32, tag="ident", name="ident")
            make_identity(nc, ident[:])
            trimask = const.tile([128, 128], F32, tag="trimask", name="trimask")
            make_causal_mask(nc, trimask[:], mask_val=-1e30)
            epsb = const.tile([128, 1], F32, tag="epsb", name="epsb")
            nc.gpsimd.memset(epsb[:], LN_EPS)

            h_t = []
            for si in range(ST):
                ht = hpool.tile([128, E], F32, tag=f"h{si}", name=f"h{si}")
                nc.sync.dma_start(out=ht[:], in_=h0[si * 128:(si + 1) * 128, :])
                h_t.append(ht)

            def emit_transposes(tag, lidx):
                """h -> hT, 48 PE transposes; copies alternate DVE/ACT."""
                hT = [htpool.tile([128, S], F32R, tag=f"ht{j}",
                                  name=f"{tag}{lidx}_{j}") for j in range(ET)]
                k = 0
                for si in range(ST):
                    for ej in range(ET):
                        tp = psum.tile([128, 128], F32, tag="small", bufs=3,
                                       name=f"{tag}p{lidx}_{si}_{ej}")
                        nc.tensor.transpose(
                            tp[:], h_t[si][:, ej * 128:(ej + 1) * 128], ident[:])
                        if k % 2 == 0:
                            nc.vector.tensor_copy(
                                hT[ej][:, si * 128:(si + 1) * 128], tp[:])
                        else:
                            nc.scalar.copy(
                                hT[ej][:, si * 128:(si + 1) * 128], tp[:])
                        k += 1
                return hT

            def emit_residual_ln(lidx, phase, items):
                """items: list of (x_tile, src_ap|None). x = LN(x + src) in place.
                Ln/Exp batched over one [128, n] tile to limit ACT table swaps."""
                n = len(items)
                vst = stat.tile([128, n], F32, tag="vst", bufs=2,
                                name=f"vst{phase}_{lidx}")
                rstd8 = stat.tile([128, n], F32, tag="rstd8", bufs=2,
                                  name=f"rstd8{phase}_{lidx}")
                nmeans = []
                for i, (xt, src_ap) in enumerate(items):
                    if src_ap is not None:
                        nc.vector.tensor_add(xt[:], xt[:], src_ap)
                    nsum = stat.tile([128, 1], F32, tag="nsum", bufs=4,
                                     name=f"ns{phase}_{lidx}_{i}")
                    nc.vector.tensor_reduce(out=nsum[:], in_=xt[:],
                                            op=ALU.add, axis=AX.X, negate=True)
                    nmean = stat.tile([128, 1], F32, tag=f"nm{i}", bufs=2,
                                      name=f"nm{phase}_{lidx}_{i}")
                    nc.vector.tensor_scalar_mul(nmean[:], nsum[:], 1.0 / E)
                    sq = ffpool.tile([128, E], F32, tag="sq", bufs=2,
                                     name=f"sq{phase}_{lidx}_{i}")
                    ssq = stat.tile([128, 1], F32, tag="ssq", bufs=4,
                                    name=f"ssq{phase}_{lidx}_{i}")
                    nc.scalar.activation(sq[:], xt[:], AF.Square,
                                         accum_out=ssq[:])
                    musq = stat.tile([128, 1], F32, tag="musq", bufs=4,
                                     name=f"mu2{phase}_{lidx}_{i}")
                    nc.vector.tensor_mul(musq[:], nmean[:], nmean[:])
                    nc.vector.tensor_scalar(out=vst[:, i:i + 1], in0=ssq[:],
                                            scalar1=1.0 / E, scalar2=musq[:],
                                            op0=ALU.mult, op1=ALU.subtract)
                    nmeans.append(nmean)
                lnv = stat.tile([128, n], F32, tag="lnv", bufs=2,
                                name=f"lnv{phase}_{lidx}")
                nc.scalar.activation(lnv[:], vst[:], AF.Ln, bias=epsb[:])
                nc.scalar.activation(rstd8[:], lnv[:], AF.Exp, scale=-0.5)
                for i, (xt, _src) in enumerate(items):
                    nb = stat.tile([128, 1], F32, tag="nb", bufs=4,
                                   name=f"nb{phase}_{lidx}_{i}")
                    nc.vector.tensor_mul(nb[:], nmeans[i][:], rstd8[:, i:i + 1])
                    nc.vector.tensor_scalar(out=xt[:], in0=xt[:],
                                            scalar1=rstd8[:, i:i + 1],
                                            scalar2=nb[:], op0=ALU.mult,
                                            op1=ALU.add)

            def emit_weights(l):
                wqk_t = wpool.tile([128, ET * QKO], F32R, tag="wqk", name=f"wqk{l}")
                nc.sync.dma_start(
                    out=wqk_t[:].rearrange("p (a n) -> p a n", a=ET),
                    in_=wqk[l].rearrange("(a p) n -> p a n", p=128))
                wv_t = wpool.tile([128, ET * VO], F32R, tag="wv", name=f"wv{l}")
                nc.sync.dma_start(
                    out=wv_t[:].rearrange("p (a n) -> p a n", a=ET),
                    in_=wv[l].rearrange("(a p) n -> p a n", p=128))
                w1_t = wpool.tile([128, ET * FFC], F32R, tag="w1", name=f"w1{l}")
                nc.sync.dma_start(
                    out=w1_t[:].rearrange("p (a n) -> p a n", a=ET),
                    in_=w1[l].rearrange("(a p) n -> p a n", p=128))
                w2_t = wpool.tile([128, 4 * E], F32R, tag="w2", name=f"w2{l}")
                nc.sync.dma_start(
                    out=w2_t[:].rearrange("p (a n) -> p a n", a=4),
                    in_=w2[l].rearrange("(a p) n -> p a n", p=128))
                return wqk_t, wv_t, w1_t, w2_t

            def emit_T(l, tgt, half, hT, k0):
                """transpose h s-tiles of one half into hT[:, half-columns]."""
                k = k0
                for si in range(half * 4, half * 4 + 4):
                    for ej in range(ET):
                        tp = psum.tile([128, 128], F32, tag="small", bufs=3,
                                       name=f"{tgt}p{l}_{si}_{ej}")
                        nc.tensor.transpose(
                            tp[:], h_t[si][:, ej * 128:(ej + 1) * 128], ident[:])
                        if k % 2 == 0:
                            nc.vector.tensor_copy(
                                hT[ej][:, si * 128:(si + 1) * 128], tp[:])
                        else:
                            nc.scalar.copy(
                                hT[ej][:, si * 128:(si + 1) * 128], tp[:])
                        k += 1

            def emit_qkv(l, half, hT, qk_t, v_sb, wqk_t, wv_t):
                sh = half
                for o in range(4):
                    ps = psum.tile([128, 512], F32, tag="big", bufs=3,
                                   name=f"qkp{l}_{o}_{sh}")
                    for ej in range(ET):
                        nc.tensor.matmul(
                            ps[:],
                            wqk_t[:, ej * QKO + o * 128: ej * QKO + (o + 1) * 128],
                            hT[ej][:, sh * 512:(sh + 1) * 512],
                            start=(ej == 0), stop=(ej == ET - 1))
                    nc.scalar.copy(qk_t[o][:, sh * 512:(sh + 1) * 512], ps[:])
                for si in range(half * 4, half * 4 + 4):
                    ps = psum.tile([128, VO], F32, tag="big", bufs=3,
                                   name=f"vp{l}_{si}")
                    for ej in range(ET):
                        nc.tensor.matmul(
                            ps[:], hT[ej][:, si * 128:(si + 1) * 128],
                            wv_t[:, ej * VO:(ej + 1) * VO],
                            start=(ej == 0), stop=(ej == ET - 1))
                    vt = vpool.tile([128, VO], F32R, tag=f"v{si}", name=f"v{l}_{si}")
                    nc.scalar.copy(vt[:], ps[:])
                    v_sb[si] = vt

            head_map = [(0, 0, 1, 0), (0, 64, 1, 64), (2, 0, 3, 0)]

            def emit_att(l, blk, qk_t, v_sb, agi, ago):
                for qi in range(4):
                    g = blk * 4 + qi
                    W = (qi + 1) * 128
                    ao_t = aopool.tile([128, VO], F32, tag="ao", name=f"ao{l}_{g}")
                    for hh in range(HPC):
                        qt, qp, kt, kp = head_map[hh]
                        Q, K = qk_t[qt], qk_t[kt]
                        if blk == 1:
                            # self-attention diagonal term, off the critical path
                            dg = psum.tile([128, 128], F32, tag="small", bufs=3,
                                           name=f"dg{l}_{hh}_{qi}")
                            nc.tensor.matmul(
                                dg[:], Q[qp:qp + 64, g * 128:(g + 1) * 128],
                                K[kp:kp + 64, NB + qi * 128:NB + W],
                                start=True, stop=True)
                            tdg = stat.tile([128, 128], F32, tag="tdg", bufs=2,
                                            name=f"tdg{l}_{hh}_{qi}")
                            nc.vector.tensor_mul(tdg[:], dg[:], ident[:])
                            dv = stat.tile([128, 1], F32, tag="dv", bufs=4,
                                           name=f"dv{l}_{hh}_{qi}")
                            nc.vector.tensor_reduce(out=dv[:], in_=tdg[:],
                                                    op=ALU.add, axis=AX.X)
                        sc = psum.tile([128, NB], F32, tag="big", bufs=3,
                                       name=f"sc{l}_{hh}_{g}")
                        nc.tensor.matmul(
                            sc[:, 0:W], Q[qp:qp + 64, g * 128:(g + 1) * 128],
                            K[kp:kp + 64, 0:W], start=True, stop=True)
                        nc.vector.tensor_add(sc[:, qi * 128:W],
                                             sc[:, qi * 128:W], trimask[:])
                        mx = stat.tile([128, 1], F32, tag="mx", bufs=6,
                                       name=f"mx{l}_{hh}_{g}")
                        # max over sc only; exp(dv+mx) may exceed 1, harmless
                        nc.vector.tensor_reduce(
                            out=mx[:], in_=sc[:, 0:W], op=ALU.max,
                            axis=AX.X, negate=True)
                        p = ppool.tile([128, NB], F32, tag="p",
                                       name=f"p{l}_{hh}_{g}")
                        rs = stat.tile([128, 1], F32, tag="rs", bufs=6,
                                       name=f"rs{l}_{hh}_{g}")
                        nc.scalar.activation(p[:, 0:W], sc[:, 0:W], AF.Exp,
                                             bias=mx[:], scale=1.0,
                                             accum_out=rs[:])
                        ri = stat.tile([128, 1], F32, tag="ri", bufs=6,
                                       name=f"ri{l}_{hh}_{g}")
                        if blk == 1:
                            pde = stat.tile([128, 1], F32, tag="pde", bufs=4,
                                            name=f"pde{l}_{hh}_{qi}")
                            nc.scalar.activation(pde[:], dv[:], AF.Exp,
                                                 bias=mx[:], scale=1.0)
                            nc.vector.tensor_add(rs[:], rs[:], pde[:])
                        nc.vector.reciprocal(ri[:], rs[:])
                        # transposes first (pipelined), then a dense matmul chain
                        pts = []
                        for mi in range(qi + 1):
                            ptp = psum.tile([128, 128], F32, tag="small",
                                            bufs=3, name=f"ptp{l}_{hh}_{g}_{mi}")
                            nc.tensor.transpose(
                                ptp[:], p[:, mi * 128:(mi + 1) * 128], ident[:])
                            pt = ptpool.tile([128, 128], F32R, tag="pt",
                                             name=f"pt{l}_{hh}_{g}_{mi}")
                            nc.vector.tensor_copy(pt[:], ptp[:])
                            pts.append(pt)
                        av = psum.tile([64, 128], F32, tag="av", bufs=2,
                                       name=f"av{l}_{hh}_{g}")
                        for mi in range(qi + 1):
                            nc.tensor.matmul(
                                av[:], v_sb[mi][:, hh * 64:(hh + 1) * 64],
                                pts[mi][:], start=(mi == 0), stop=(mi == qi),
                                skip_group_check=True)
                        avs = avspool.tile([64, 128], F32, tag="avs",
                                           name=f"avs{l}_{hh}_{g}")
                        nc.vector.tensor_copy(avs[:], av[:])
                        tph = psum.tile([128, 64], F32, tag="av", bufs=2,
                                        name=f"aotp{l}_{g}_{hh}")
                        nc.tensor.transpose(tph[:], avs[:], ident[0:64, 0:64])
                        # evict + 1/rowsum scale in one op
                        nc.vector.tensor_scalar_mul(
                            ao_t[:, hh * 64:(hh + 1) * 64], tph[:], ri[:])
                        if blk == 1:
                            pdn = stat.tile([128, 1], F32, tag="pdn", bufs=4,
                                            name=f"pdn{l}_{hh}_{qi}")
                            nc.vector.tensor_mul(pdn[:], pde[:], ri[:])
                            # ao += v * pdn in one op
                            nc.vector.scalar_tensor_tensor(
                                out=ao_t[:, hh * 64:(hh + 1) * 64],
                                in0=v_sb[g][:, hh * 64:(hh + 1) * 64].bitcast(F32),
                                scalar=pdn[:],
                                in1=ao_t[:, hh * 64:(hh + 1) * 64],
                                op0=ALU.mult, op1=ALU.add)
                    nc.sync.dma_start(out=agi[qi * 128:(qi + 1) * 128, :],
                                      in_=ao_t[:])
                nc.gpsimd.collective_compute(
                    "AllGather", ALU.bypass, replica_groups=RG,
                    ins=[agi[:].opt()], outs=[ago[:].opt()])

            def emit_ln1_t2_ffn1(l, half, ago, hT2, hid, w1_t):
                items = []
                for si in range(half * 4, half * 4 + 4):
                    aof = ffpool.tile([128, E], F32, tag="aof", bufs=6,
                                      name=f"aof{l}_{si}")
                    nc.sync.dma_start(
                        out=aof[:].rearrange("s (r v) -> s r v", r=4),
                        in_=ago.rearrange("r s v -> s r v")[
                            (si % 4) * 128:(si % 4 + 1) * 128])
                    items.append((h_t[si], aof[:]))
                emit_residual_ln(l, f"a{half}", items)
                emit_T(l, "hU", half, hT2, half * 24)
                for ft in range(4):
                    ps = psum.tile([128, 512], F32, tag="big", bufs=3,
                                   name=f"f1p{l}_{ft}_{half}")
                    for ej in range(ET):
                        nc.tensor.matmul(
                            ps[:],
                            w1_t[:, ej * FFC + ft * 128: ej * FFC + (ft + 1) * 128],
                            hT2[ej][:, half * 512:(half + 1) * 512],
                            start=(ej == 0), stop=(ej == ET - 1))
                    nc.scalar.activation(hid[ft][:, half * 512:(half + 1) * 512],
                                         ps[:], AF.Gelu)

            def emit_ff2(l, half, hid, w2_t, ari, aro):
                for si in range(half * 4, half * 4 + 4):
                    ff_t = ffpool.tile([128, E], F32, tag="fft",
                                       name=f"fft{l}_{si}")
                    pa = psum.tile([128, 512], F32, tag="big", bufs=3,
                                   name=f"f2a{l}_{si}")
                    for ft in range(4):
                        nc.tensor.matmul(
                            pa[:], hid[ft][:, si * 128:(si + 1) * 128],
                            w2_t[:, ft * E: ft * E + 512],
                            start=(ft == 0), stop=(ft == 3))
                    nc.scalar.copy(ff_t[:, 0:512], pa[:])
                    pb = psum.tile([128, 256], F32, tag="small", bufs=3,
                                   name=f"f2b{l}_{si}")
                    for ft in range(4):
                        nc.tensor.matmul(
                            pb[:], hid[ft][:, si * 128:(si + 1) * 128],
                            w2_t[:, ft * E + 512:(ft + 1) * E],
                            start=(ft == 0), stop=(ft == 3))
                    nc.vector.tensor_copy(ff_t[:, 512:768], pb[:])
                    # fold h/4 so the AllReduce sum includes the residual
                    nc.vector.scalar_tensor_tensor(
                        out=ff_t[:], in0=h_t[si][:], scalar=0.25, in1=ff_t[:],
                        op0=ALU.mult, op1=ALU.add)
                    nc.sync.dma_start(
                        out=ari[(si % 4) * 128:(si % 4 + 1) * 128, :],
                        in_=ff_t[:])
                nc.gpsimd.collective_compute(
                    "AllReduce", ALU.add, replica_groups=RG,
                    ins=[ari[:].opt()], outs=[aro[:].opt()])

            def emit_ln2(l, half, aro):
                items = []
                for si in range(half * 4, half * 4 + 4):
                    nc.sync.dma_start(
                        out=h_t[si][:],
                        in_=aro[(si % 4) * 128:(si % 4 + 1) * 128, :])
                    items.append((h_t[si], None))
                emit_residual_ln(l, f"b{half}", items)

            pend_l2b = [None]  # deferred L2B emission state
            for l in range(L):
                wqk_t, wv_t, w1_t, w2_t = emit_weights(l)
                hT = [htpool.tile([128, S], F32R, tag=f"ht{j}", name=f"hT{l}_{j}")
                      for j in range(ET)]
                qk_t = [qkpool.tile([128, S], F32R, tag=f"qk{o}", name=f"qk{l}_{o}")
                        for o in range(4)]
                hT2 = [htpool.tile([128, S], F32R, tag=f"ht{j}", name=f"hU{l}_{j}")
                       for j in range(ET)]
                hid = [hidpool.tile([128, S], F32R, tag=f"hid{t}",
                                    name=f"hid{l}_{t}") for t in range(4)]
                v_sb = [None] * ST
                agi_b = [dram.tile([NB, VO], F32, tag=f"agi{b}", name=f"agi{l}_{b}")
                         for b in range(2)]
                ago_b = [dram.tile([4, NB, VO], F32, tag=f"ago{b}",
                                   name=f"ago{l}_{b}") for b in range(2)]
                ari_b = [dram.tile([NB, E], F32, tag=f"ari{b}", name=f"ari{l}_{b}")
                         for b in range(2)]
                aro_b = [dram.tile([NB, E], F32, tag=f"aro{b}", name=f"aro{l}_{b}")
                         for b in range(2)]

                # A/B streams interleaved + cross-layer software pipelining:
                # L2B(l-1) is emitted after ATTA(l) so the in-order engine
                # queues never park on AllReduce-B while A-work is available.
                with nc.named_scope(f"TQA{l}"):
                    emit_T(l, "hT", 0, hT, 0)
                    emit_qkv(l, 0, hT, qk_t, v_sb, wqk_t, wv_t)
                with nc.named_scope(f"ATTA{l}"):
                    emit_att(l, 0, qk_t, v_sb, agi_b[0], ago_b[0])
                if pend_l2b[0] is not None:
                    lp, aro_p = pend_l2b[0]
                    with nc.named_scope(f"L2B{lp}"):
                        emit_ln2(lp, 1, aro_p)
                    pend_l2b[0] = None
                with nc.named_scope(f"TQB{l}"):
                    emit_T(l, "hT", 1, hT, 24)
                    emit_qkv(l, 1, hT, qk_t, v_sb, wqk_t, wv_t)
                with nc.named_scope(f"ATTB{l}"):
                    emit_att(l, 1, qk_t, v_sb, agi_b[1], ago_b[1])
                with nc.named_scope(f"FNA{l}"):
                    emit_ln1_t2_ffn1(l, 0, ago_b[0], hT2, hid, w1_t)
                    emit_ff2(l, 0, hid, w2_t, ari_b[0], aro_b[0])
                with nc.named_scope(f"FNB{l}"):
                    emit_ln1_t2_ffn1(l, 1, ago_b[1], hT2, hid, w1_t)
                    emit_ff2(l, 1, hid, w2_t, ari_b[1], aro_b[1])
                with nc.named_scope(f"L2A{l}"):
                    emit_ln2(l, 0, aro_b[0])
                pend_l2b[0] = (l, aro_b[1])
            lp, aro_p = pend_l2b[0]
            with nc.named_scope(f"L2B{lp}"):
                emit_ln2(lp, 1, aro_p)

            # ---- final LN -> out ----
            emit_residual_ln(L, "f", [(h_t[si], None) for si in range(ST)])
            for si in range(ST):
                nc.sync.dma_start(out=out[si * 128:(si + 1) * 128, :],
                                  in_=h_t[si][:])

    nc.compile()
    return nc


def _get_nc():
    global _NC_CACHE
    if _NC_CACHE is None:
        _NC_CACHE = _build()
    return _NC_CACHE


def _sinusoidal_pe(max_len, d):
    pos = np.arange(max_len)[:, None]
    div = np.exp(np.arange(0, d, 2) * (-np.log(10000.0) / d))
    pe = np.zeros((max_len, d), np.float32)
    pe[:, 0::2] = np.sin(pos * div)
    pe[:, 1::2] = np.cos(pos * div)
    return pe


def kernel(x, padding_mask, thought_pe, Wqkv, bqkv, W1, b1, W2, b2,
           ln1_w, ln1_b, ln2_w, ln2_b, lnf_w, lnf_b,
           thoughts_taken, real_token_count, **_unused):
    global LAST_RESULT
    x = np.asarray(x, np.float32)
    thought_pe = np.asarray(thought_pe, np.float32)
    Wqkv = np.asarray(Wqkv, np.float32)
    W1 = np.asarray(W1, np.float32)
    W2 = np.asarray(W2, np.float32)
    nt = int(thoughts_taken) + 1
    rtc = int(real_token_count)
    B = x.shape[0]
    assert nt == 2 and rtc * nt == S and B == 2, (nt, rtc, B)
    assert not (np.any(np.asarray(bqkv)) or np.any(np.asarray(b1))
                or np.any(np.asarray(b2)))
    for w_, b_ in ((ln1_w, ln1_b), (ln2_w, ln2_b), (lnf_w, lnf_b)):
        assert np.all(np.asarray(w_) == 1.0) and not np.any(np.asarray(b_))

    # dual positional encoding (host, matches reference fp32 order of adds)
    pe = _sinusoidal_pe(S, E)
    h = x[:, : rtc * nt].reshape(B, rtc, nt, E)
    h = h + pe[:rtc][None, :, None, :] + thought_pe[:nt][None, None, :, :]
    h = h.reshape(B, S, E)

    # de-interleave: block A = thought-0 rows (even), block B = thought-1 (odd)
    perm = np.concatenate([np.arange(0, S, 2), np.arange(1, S, 2)])
    inv = np.argsort(perm)
    hp = np.ascontiguousarray(h[:, perm])

    in_maps = []
    for c in range(8):
        b, r = divmod(c, 4)
        wq = Wqkv[:, r * VO:(r + 1) * VO, :] * np.float32(1.0 / np.sqrt(D))
        wk = Wqkv[:, E + r * VO: E + (r + 1) * VO, :]
        wvs = Wqkv[:, 2 * E + r * VO: 2 * E + (r + 1) * VO, :]
        # feature order [Q0,Q1 | K0,K1 | Q2,K2 | K2,Q2]: per-head Q/K pairs
        # land at matching SBUF partition bases (matmul requirement)
        q0, q1, q2 = wq[:, 0:64], wq[:, 64:128], wq[:, 128:192]
        k0, k1, k2 = wk[:, 0:64], wk[:, 64:128], wk[:, 128:192]
        wqk_feats = np.concatenate([q0, q1, k0, k1, q2, k2, k2, q2], axis=1)
        in_maps.append({
            "h0": hp[b],
            "wqk": np.ascontiguousarray(wqk_feats.transpose(0, 2, 1)),
            "wv": np.ascontiguousarray(wvs.transpose(0, 2, 1)),
            "w1": np.ascontiguousarray(
                W1[:, r * FFC:(r + 1) * FFC, :].transpose(0, 2, 1)),
            "w2": np.ascontiguousarray(
                W2[:, :, r * FFC:(r + 1) * FFC].transpose(0, 2, 1)),
        })

    res = run_bass_kernel_spmd(_get_nc(), in_maps, list(range(8)))
    LAST_RESULT = res
    outp = np.empty((B, S, E), np.float32)
    outp[0] = res.results[0]["out"][inv]
    outp[1] = res.results[4]["out"][inv]
    return outp



# revision 2
# speedup vs baseline: 1.0404x; 1.0404x over previous
"""Trainium2 Bass kernel for nn_CausalTransformer_81776177316304.

Strategy: DP-2 over batch x sequence-parallel-4 within each group.

Core r of a group owns A-rows [128r,128r+128) (thought-0, de-interleaved)
and B-rows [512+128r, 512+128r+128) (thought-1). FFN / LayerNorm / QKV are
row-local, so the only collective is one small AllGather of (K^T, V) for the
A half (bf16) per layer; the B-diagonal attention term uses local K/V.
Per-core causal extents are encoded in a host-built additive mask so all 8
cores run one SPMD program. Matmul inputs are bf16 (weights cast on host),
accumulation and the residual stream stay fp32.
"""

import numpy as np
import ml_dtypes

import concourse.bass as bass
import concourse.mybir as mybir
import concourse.tile as tile
from concourse import bacc
from concourse.bass_utils import run_bass_kernel_spmd
from concourse.masks import make_identity

F32 = mybir.dt.float32
BF16 = mybir.dt.bfloat16
AF = mybir.ActivationFunctionType
ALU = mybir.AluOpType
AX = mybir.AxisListType

S, E, H, L, FF, D = 1024, 768, 12, 4, 2048, 64
NB = S // 2                  # 512: A/B block size
R = 256                      # rows owned per core (128 A + 128 B)
ET = E // 128                # 6 e-tiles
FT = FF // 128               # 16 ff-tiles
LN_EPS = 1e-5
RG = [[0, 1, 2, 3], [4, 5, 6, 7]]

_NC_CACHE = None
LAST_RESULT = None


def _build():
    nc = bacc.Bacc("TRN2", target_bir_lowering=False, debug=False, num_devices=8)
    h0 = nc.dram_tensor("h0", [R, E], F32, kind="ExternalInput")
    maskd = nc.dram_tensor("maskd", [128, NB], F32, kind="ExternalInput")
    wqkv = nc.dram_tensor("wqkv", [L, E, 3 * E], BF16, kind="ExternalInput")
    w1 = nc.dram_tensor("w1", [L, E, FF], BF16, kind="ExternalInput")
    w2 = nc.dram_tensor("w2", [L, FF, E], BF16, kind="ExternalInput")
    out = nc.dram_tensor("out", [R, E], F32, kind="ExternalOutput")

    from contextlib import ExitStack
    with tile.TileContext(nc) as tc:
        with ExitStack() as ctx:
            const = ctx.enter_context(tc.tile_pool(name="const", bufs=1))
            hpool = ctx.enter_context(tc.tile_pool(name="hpool", bufs=1))
            htpool = ctx.enter_context(tc.tile_pool(name="htpool", bufs=1))
            wqpool = ctx.enter_context(tc.tile_pool(name="wqpool", bufs=2))
            wfpool = ctx.enter_context(tc.tile_pool(name="wfpool", bufs=1))
            qkpool = ctx.enter_context(tc.tile_pool(name="qkpool", bufs=1))
            vpool = ctx.enter_context(tc.tile_pool(name="vpool", bufs=1))
            gpool = ctx.enter_context(tc.tile_pool(name="gpool", bufs=2))
            ppool = ctx.enter_context(tc.tile_pool(name="ppool", bufs=3))
            ptpool = ctx.enter_context(tc.tile_pool(name="ptpool", bufs=8))
            aopool = ctx.enter_context(tc.tile_pool(name="aopool", bufs=2))
            ffpool = ctx.enter_context(tc.tile_pool(name="ffpool", bufs=4))
            hidpool = ctx.enter_context(tc.tile_pool(name="hidpool", bufs=1))
            stat = ctx.enter_context(tc.tile_pool(name="stat", bufs=4))
            psum = ctx.enter_context(tc.tile_pool(name="psum", bufs=2, space="PSUM"))
            dram = ctx.enter_context(tc.tile_pool(name="dram", bufs=2, space="DRAM"))

            ident = const.tile([128, 128], F32, tag="ident", name="ident")
            make_identity(nc, ident[:])
            epsb = const.tile([128, 1], F32, tag="epsb", name="epsb")
            nc.gpsimd.memset(epsb[:], LN_EPS)
            mask_t = const.tile([128, NB], F32, tag="mask", name="mask")
            nc.sync.dma_start(out=mask_t[:], in_=maskd[:, :])

            # residual stream: 2 row-tiles of [128, E] fp32
            h_t = []
            for t in range(2):
                ht = hpool.tile([128, E], F32, tag=f"h{t}", name=f"h{t}")
                nc.sync.dma_start(out=ht[:], in_=h0[t * 128:(t + 1) * 128, :])
                h_t.append(ht)

            def emit_T(l, tag, dst):
                """h_t -> dst[ej][:, 0:256] bf16 transposes (12 PE + evicts)."""
                k = 0
                for t in range(2):
                    for ej in range(ET):
                        tp = psum.tile([128, 128], F32, tag="small", bufs=3,
                                       name=f"{tag}p{l}_{t}_{ej}")
                        nc.tensor.transpose(
                            tp[:], h_t[t][:, ej * 128:(ej + 1) * 128], ident[:])
                        if k % 2 == 0:
                            nc.vector.tensor_copy(
                                dst[ej][:, t * 128:(t + 1) * 128], tp[:])
                        else:
                            nc.scalar.copy(
                                dst[ej][:, t * 128:(t + 1) * 128], tp[:])
                        k += 1

            def emit_residual_ln(lidx, phase, items):
                """items: list of (x_tile, src_ap|None). x = LN(x + src) in place."""
                n = len(items)
                vst = stat.tile([128, n], F32, tag="vst", bufs=2,
                                name=f"vst{phase}_{lidx}")
                rstd8 = stat.tile([128, n], F32, tag="rstd8", bufs=2,
                                  name=f"rstd8{phase}_{lidx}")
                nmeans = []
                for i, (xt, src_ap) in enumerate(items):
                    if src_ap is not None:
                        nc.vector.tensor_add(xt[:], xt[:], src_ap)
                    nsum = stat.tile([128, 1], F32, tag="nsum", bufs=4,
                                     name=f"ns{phase}_{lidx}_{i}")
                    nc.vector.tensor_reduce(out=nsum[:], in_=xt[:],
                                            op=ALU.add, axis=AX.X, negate=True)
                    nmean = stat.tile([128, 1], F32, tag=f"nm{i}", bufs=2,
                                      name=f"nm{phase}_{lidx}_{i}")
                    nc.vector.tensor_scalar_mul(nmean[:], nsum[:], 1.0 / E)
                    sq = ffpool.tile([128, E], F32, tag="sq", bufs=2,
                                     name=f"sq{phase}_{lidx}_{i}")
                    ssq = stat.tile([128, 1], F32, tag="ssq", bufs=4,
                                    name=f"ssq{phase}_{lidx}_{i}")
                    nc.scalar.activation(sq[:], xt[:], AF.Square,
                                         accum_out=ssq[:])
                    musq = stat.tile([128, 1], F32, tag="musq", bufs=4,
                                     name=f"mu2{phase}_{lidx}_{i}")
                    nc.vector.tensor_mul(musq[:], nmean[:], nmean[:])
                    nc.vector.tensor_scalar(out=vst[:, i:i + 1], in0=ssq[:],
                                            scalar1=1.0 / E, scalar2=musq[:],
                                            op0=ALU.mult, op1=ALU.subtract)
                    nmeans.append(nmean)
                lnv = stat.tile([128, n], F32, tag="lnv", bufs=2,
                                name=f"lnv{phase}_{lidx}")
                nc.scalar.activation(lnv[:], vst[:], AF.Ln, bias=epsb[:])
                nc.scalar.activation(rstd8[:], lnv[:], AF.Exp, scale=-0.5)
                for i, (xt, _src) in enumerate(items):
                    nb = stat.tile([128, 1], F32, tag="nb", bufs=4,
                                   name=f"nb{phase}_{lidx}_{i}")
                    nc.vector.tensor_mul(nb[:], nmeans[i][:], rstd8[:, i:i + 1])
                    nc.vector.tensor_scalar(out=xt[:], in0=xt[:],
                                            scalar1=rstd8[:, i:i + 1],
                                            scalar2=nb[:], op0=ALU.mult,
                                            op1=ALU.add)

            for l in range(L):
                # ---- weight DMA (wqkv double-buffered across layers) ----
                wqkv_t = wqpool.tile([128, ET * 3 * E], BF16, tag="wqkv",
                                     name=f"wqkv{l}")
                nc.sync.dma_start(
                    out=wqkv_t[:].rearrange("p (a n) -> p a n", a=ET),
                    in_=wqkv[l].rearrange("(a p) n -> p a n", p=128))
                w1_t = wfpool.tile([128, ET * FF], BF16, tag="w1", name=f"w1{l}")
                nc.sync.dma_start(
                    out=w1_t[:].rearrange("p (a n) -> p a n", a=ET),
                    in_=w1[l].rearrange("(a p) n -> p a n", p=128))
                w2_t = wfpool.tile([128, FT * E], BF16, tag="w2", name=f"w2{l}")
                nc.sync.dma_start(
                    out=w2_t[:].rearrange("p (a n) -> p a n", a=FT),
                    in_=w2[l].rearrange("(a p) n -> p a n", p=128))

                hT = [htpool.tile([128, R], BF16, tag=f"ht{j}", name=f"hT{l}_{j}")
                      for j in range(ET)]
                hT2 = [htpool.tile([128, R], BF16, tag=f"hu{j}", name=f"hU{l}_{j}")
                       for j in range(ET)]
                qT = [qkpool.tile([128, R], BF16, tag=f"q{j}", name=f"qT{l}_{j}")
                      for j in range(ET)]
                kT = [qkpool.tile([128, R], BF16, tag=f"k{j}", name=f"kT{l}_{j}")
                      for j in range(ET)]
                vOwn = [vpool.tile([128, E], BF16, tag=f"vo{t}", name=f"vO{l}_{t}")
                        for t in range(2)]
                kTA = [gpool.tile([128, NB], BF16, tag=f"ga{j}", name=f"kA{l}_{j}")
                       for j in range(ET)]
                vA = [gpool.tile([128, E], BF16, tag=f"gv{c}", name=f"vA{l}_{c}")
                      for c in range(4)]
                hidT = [hidpool.tile([128, R], BF16, tag=f"hd{f}",
                                     name=f"hid{l}_{f}") for f in range(FT)]
                ao_t = [aopool.tile([128, E], F32, tag=f"ao{t}", name=f"ao{l}_{t}")
                        for t in range(2)]
                agi = dram.tile([2, E, 128], BF16, tag="agi", name=f"agi{l}")
                ago = dram.tile([4, 2, E, 128], BF16, tag="ago", name=f"ago{l}")

                with nc.named_scope(f"TKV{l}"):
                    emit_T(l, "hT", hT)
                    # K^T for own rows (both tiles; A half feeds the gather)
                    for f in range(ET):
                        ps = psum.tile([128, R], F32, tag="big", bufs=3,
                                       name=f"kp{l}_{f}")
                        for ej in range(ET):
                            nc.tensor.matmul(
                                ps[:],
                                wqkv_t[:, ej * 3 * E + E + f * 128:
                                       ej * 3 * E + E + (f + 1) * 128],
                                hT[ej][:, :],
                                start=(ej == 0), stop=(ej == ET - 1))
                        nc.scalar.copy(kT[f][:], ps[:])
                    # V for own rows (row layout), both tiles
                    for t in range(2):
                        for half, w in ((0, 512), (1, 256)):
                            ps = psum.tile([128, w], F32, tag="big", bufs=3,
                                           name=f"vp{l}_{t}_{half}")
                            for ej in range(ET):
                                nc.tensor.matmul(
                                    ps[:],
                                    hT[ej][:, t * 128:(t + 1) * 128],
                                    wqkv_t[:, ej * 3 * E + 2 * E + half * 512:
                                           ej * 3 * E + 2 * E + half * 512 + w],
                                    start=(ej == 0), stop=(ej == ET - 1))
                            nc.vector.tensor_copy(
                                vOwn[t][:, half * 512:half * 512 + w], ps[:])
                    # stage A-half K^T/V and kick the gather
                    for f in range(ET):
                        nc.sync.dma_start(out=agi[0, f * 128:(f + 1) * 128, :],
                                          in_=kT[f][:, 0:128])
                    nc.sync.dma_start(
                        out=agi[1].rearrange("(p q) y -> p (q y)", p=128),
                        in_=vOwn[0][:, :])
                    nc.gpsimd.collective_compute(
                        "AllGather", ALU.bypass, replica_groups=RG,
                        ins=[agi[:].opt()], outs=[ago[:].opt()])

                with nc.named_scope(f"QD{l}"):
                    # Q^T own rows (overlaps the gather)
                    for f in range(ET):
                        ps = psum.tile([128, R], F32, tag="big", bufs=3,
                                       name=f"qp{l}_{f}")
                        for ej in range(ET):
                            nc.tensor.matmul(
                                ps[:],
                                wqkv_t[:, ej * 3 * E + f * 128:
                                       ej * 3 * E + (f + 1) * 128],
                                hT[ej][:, :],
                                start=(ej == 0), stop=(ej == ET - 1))
                        nc.scalar.copy(qT[f][:], ps[:])
                    # B-diagonal raw scores per head: dv[h] = diag(Qb^T Kb)
                    dvs = []
                    for h in range(H):
                        f, off = h // 2, (h % 2) * 64
                        dg = psum.tile([128, 128], F32, tag="small", bufs=3,
                                       name=f"dg{l}_{h}")
                        nc.tensor.matmul(
                            dg[:], qT[f][off:off + 64, 128:256],
                            kT[f][off:off + 64, 128:256], start=True, stop=True)
                        tdg = stat.tile([128, 128], F32, tag="tdg", bufs=2,
                                        name=f"tdg{l}_{h}")
                        nc.vector.tensor_mul(tdg[:], dg[:], ident[:])
                        dv = stat.tile([128, 1], F32, tag=f"dv{h}", bufs=1,
                                       name=f"dv{l}_{h}")
                        nc.vector.tensor_reduce(out=dv[:], in_=tdg[:],
                                                op=ALU.add, axis=AX.X)
                        dvs.append(dv)
                    # pull gathered K^T_A / V_A into SBUF
                    for f in range(ET):
                        for r in range(4):
                            nc.sync.dma_start(
                                out=kTA[f][:, r * 128:(r + 1) * 128],
                                in_=ago[r, 0, f * 128:(f + 1) * 128, :])
                    for c in range(4):
                        nc.sync.dma_start(
                            out=vA[c][:],
                            in_=ago[c, 1].rearrange("(p q) y -> p (q y)", p=128))

                with nc.named_scope(f"ATT{l}"):
                    for h in range(H):
                        f, off = h // 2, (h % 2) * 64
                        for qt in range(2):
                            sc = psum.tile([128, NB], F32, tag="big", bufs=3,
                                           name=f"sc{l}_{h}_{qt}")
                            nc.tensor.matmul(
                                sc[:], qT[f][off:off + 64, qt * 128:(qt + 1) * 128],
                                kTA[f][off:off + 64, :], start=True, stop=True)
                            nc.vector.tensor_add(sc[:], sc[:], mask_t[:])
                            mx = stat.tile([128, 1], F32, tag="mx", bufs=6,
                                           name=f"mx{l}_{h}_{qt}")
                            nc.vector.tensor_reduce(out=mx[:], in_=sc[:],
                                                    op=ALU.max, axis=AX.X,
                                                    negate=True)
                            p = ppool.tile([128, NB], F32, tag="p",
                                           name=f"p{l}_{h}_{qt}")
                            rs = stat.tile([128, 1], F32, tag="rs", bufs=6,
                                           name=f"rs{l}_{h}_{qt}")
                            nc.scalar.activation(p[:], sc[:], AF.Exp,
                                                 bias=mx[:], scale=1.0,
                                                 accum_out=rs[:])
                            ri = stat.tile([128, 1], F32, tag="ri", bufs=6,
                                           name=f"ri{l}_{h}_{qt}")
                            if qt == 1:
                                pde = stat.tile([128, 1], F32, tag="pde", bufs=4,
                                                name=f"pde{l}_{h}")
                                nc.scalar.activation(pde[:], dvs[h][:], AF.Exp,
                                                     bias=mx[:], scale=1.0)
                                nc.vector.tensor_add(rs[:], rs[:], pde[:])
                            nc.vector.reciprocal(ri[:], rs[:])
                            pts = []
                            for mi in range(4):
                                ptp = psum.tile([128, 128], F32, tag="small",
                                                bufs=3, name=f"ptp{l}_{h}_{qt}_{mi}")
                                nc.tensor.transpose(
                                    ptp[:], p[:, mi * 128:(mi + 1) * 128],
                                    ident[:])
                                pt = ptpool.tile([128, 128], BF16, tag="pt",
                                                 name=f"pt{l}_{h}_{qt}_{mi}")
                                if mi % 2 == 0:
                                    nc.vector.tensor_copy(pt[:], ptp[:])
                                else:
                                    nc.scalar.copy(pt[:], ptp[:])
                                pts.append(pt)
                            av = psum.tile([128, 64], F32, tag="av", bufs=2,
                                           name=f"av{l}_{h}_{qt}")
                            for mi in range(4):
                                nc.tensor.matmul(
                                    av[:], pts[mi][:],
                                    vA[mi][:, h * 64:(h + 1) * 64],
                                    start=(mi == 0), stop=(mi == 3),
                                    skip_group_check=True)
                            nc.vector.tensor_scalar_mul(
                                ao_t[qt][:, h * 64:(h + 1) * 64], av[:], ri[:])
                            if qt == 1:
                                pdn = stat.tile([128, 1], F32, tag="pdn", bufs=4,
                                                name=f"pdn{l}_{h}")
                                nc.vector.tensor_mul(pdn[:], pde[:], ri[:])
                                nc.vector.scalar_tensor_tensor(
                                    out=ao_t[1][:, h * 64:(h + 1) * 64],
                                    in0=vOwn[1][:, h * 64:(h + 1) * 64],
                                    scalar=pdn[:],
                                    in1=ao_t[1][:, h * 64:(h + 1) * 64],
                                    op0=ALU.mult, op1=ALU.add)

                with nc.named_scope(f"FFN{l}"):
                    emit_residual_ln(l, "a", [(h_t[t], ao_t[t][:])
                                              for t in range(2)])
                    emit_T(l, "hU", hT2)
                    for f in range(FT):
                        ps = psum.tile([128, R], F32, tag="big", bufs=3,
                                       name=f"f1p{l}_{f}")
                        for ej in range(ET):
                            nc.tensor.matmul(
                                ps[:],
                                w1_t[:, ej * FF + f * 128:ej * FF + (f + 1) * 128],
                                hT2[ej][:, :],
                                start=(ej == 0), stop=(ej == ET - 1))
                        nc.scalar.activation(hidT[f][:], ps[:], AF.Gelu)
                    ffs = []
                    for t in range(2):
                        ff_t = ffpool.tile([128, E], F32, tag="fft",
                                           name=f"fft{l}_{t}")
                        for half, w in ((0, 512), (1, 256)):
                            ps = psum.tile([128, w], F32, tag="big", bufs=3,
                                           name=f"f2p{l}_{t}_{half}")
                            for f in range(FT):
                                nc.tensor.matmul(
                                    ps[:],
                                    hidT[f][:, t * 128:(t + 1) * 128],
                                    w2_t[:, f * E + half * 512:
                                         f * E + half * 512 + w],
                                    start=(f == 0), stop=(f == FT - 1))
                            if half == 0:
                                nc.vector.tensor_copy(
                                    ff_t[:, 0:512], ps[:])
                            else:
                                nc.scalar.copy(ff_t[:, 512:768], ps[:])
                        ffs.append(ff_t)
                    emit_residual_ln(l, "b", [(h_t[t], ffs[t][:])
                                              for t in range(2)])

            # ---- final LN -> out ----
            emit_residual_ln(L, "f", [(h_t[t], None) for t in range(2)])
            for t in range(2):
                nc.sync.dma_start(out=out[t * 128:(t + 1) * 128, :],
                                  in_=h_t[t][:])

    nc.compile()
    return nc


def _get_nc():
    global _NC_CACHE
    if _NC_CACHE is None:
        _NC_CACHE = _build()
    return _NC_CACHE


def _sinusoidal_pe(max_len, d):
    pos = np.arange(max_len)[:, None]
    div = np.exp(np.arange(0, d, 2) * (-np.log(10000.0) / d))
    pe = np.zeros((max_len, d), np.float32)
    pe[:, 0::2] = np.sin(pos * div)
    pe[:, 1::2] = np.cos(pos * div)
    return pe


def kernel(x, padding_mask, thought_pe, Wqkv, bqkv, W1, b1, W2, b2,
           ln1_w, ln1_b, ln2_w, ln2_b, lnf_w, lnf_b,
           thoughts_taken, real_token_count, **_unused):
    global LAST_RESULT
    x = np.asarray(x, np.float32)
    thought_pe = np.asarray(thought_pe, np.float32)
    Wqkv = np.asarray(Wqkv, np.float32)
    W1 = np.asarray(W1, np.float32)
    W2 = np.asarray(W2, np.float32)
    nt = int(thoughts_taken) + 1
    rtc = int(real_token_count)
    B = x.shape[0]
    assert nt == 2 and rtc * nt == S and B == 2, (nt, rtc, B)
    assert not (np.any(np.asarray(bqkv)) or np.any(np.asarray(b1))
                or np.any(np.asarray(b2)))
    for w_, b_ in ((ln1_w, ln1_b), (ln2_w, ln2_b), (lnf_w, lnf_b)):
        assert np.all(np.asarray(w_) == 1.0) and not np.any(np.asarray(b_))

    # dual positional encoding (host, matches reference fp32 order of adds)
    pe = _sinusoidal_pe(S, E)
    h = x[:, : rtc * nt].reshape(B, rtc, nt, E)
    h = h + pe[:rtc][None, :, None, :] + thought_pe[:nt][None, None, :, :]
    h = h.reshape(B, S, E)

    # de-interleave: block A = thought-0 rows (even), block B = thought-1 (odd)
    perm = np.concatenate([np.arange(0, S, 2), np.arange(1, S, 2)])
    inv = np.argsort(perm)
    hp = np.ascontiguousarray(h[:, perm])

    bf = ml_dtypes.bfloat16
    # weight layouts: all W^T [E, feat] slabs, Q scaled by 1/sqrt(D)
    wq = Wqkv[:, 0:E, :] * np.float32(1.0 / np.sqrt(D))
    wk = Wqkv[:, E:2 * E, :]
    wv = Wqkv[:, 2 * E:3 * E, :]
    wqkv_h = np.ascontiguousarray(
        np.concatenate([wq, wk, wv], axis=1).transpose(0, 2, 1)).astype(bf)
    w1_h = np.ascontiguousarray(W1.transpose(0, 2, 1)).astype(bf)   # [L,E,FF]
    w2_h = np.ascontiguousarray(W2.transpose(0, 2, 1)).astype(bf)   # [L,FF,E]

    in_maps = []
    for c in range(8):
        b, r = divmod(c, 4)
        rows = np.concatenate([hp[b, 128 * r:128 * (r + 1)],
                               hp[b, NB + 128 * r:NB + 128 * (r + 1)]])
        # causal extent mask over the 512 A keys for row-tile r
        i = 128 * r + np.arange(128)[:, None]
        j = np.arange(NB)[None, :]
        mask = np.where(j <= i, 0.0, -1e30).astype(np.float32)
        in_maps.append({
            "h0": np.ascontiguousarray(rows),
            "maskd": mask,
            "wqkv": wqkv_h,
            "w1": w1_h,
            "w2": w2_h,
        })

    res = run_bass_kernel_spmd(_get_nc(), in_maps, list(range(8)))
    LAST_RESULT = res
    outp = np.empty((B, S, E), np.float32)
    for b in range(B):
        full = np.empty((S, E), np.float32)
        for r in range(4):
            o = res.results[b * 4 + r]["out"]
            full[128 * r:128 * (r + 1)] = o[0:128]
            full[NB + 128 * r:NB + 128 * (r + 1)] = o[128:256]
        outp[b] = full[inv]
    return outp
